# revision 26
# baseline (speedup 1.0000x reference)
"""Distributed Bass kernel for nn_AdaGNN (2-layer GAT + MLP heads + CE losses).

Strategy (8 NeuronCores, SPMD):
  - Nodes assigned to 8 cores x NT tiles of 128 by a load-balancing packer
    (equal edge counts per tile). Output is permutation invariant. Tiles are
    renumbered into schedule (finalize) order so every per-tile cache slice
    is contiguous and table rows are written sequentially.
  - Per layer: dense per-node transform producing a 128-wide (256B) gather-table
    row [feat(64) | a_src(8) | a_dst(8) | pad] bf16 -> AllGather -> per-edge
    dma_gather (int16 indices; 4 source windows of TBL/4 rows each) ->
    per-edge a_dst via batched PE transposes of the one-hots + 8-col matmuls
    -> segment softmax via exp (value ranges are small; max-subtraction
    unnecessary) -> weighted segment-sum via one-hot matmuls on TensorE ->
    batched per-group normalize.
  - Edge chunks of 128 are keyed (tile, window, q) with a per-tile/window
    chunk schedule shared by all cores (SPMD-static); tiles are grouped into
    batches of identical schedule vectors; all per-edge tensors are laid out
    window-major so per-batch vector ops are single instructions.
  - Layer 2 aggregates per-head-weighted 64-dim inputs (512-wide messages) and
    applies the reshuffled W2 (mean over heads folded in) after aggregation.
  - Finalize runs per batch: per-tile PSUM->SBUF copy, then group-batched
    softmax-normalization / layer-2 table build / MLP heads / CE.
  - Partial CE sums AllReduced; final scalar computed on device.
"""

import math
import numpy as np
import ml_dtypes

import concourse.bass as bass
import concourse.tile as tile
from concourse import mybir
from concourse.bacc import Bacc
from concourse.bass_utils import run_bass_kernel_spmd

BF16 = mybir.dt.bfloat16
F32 = mybir.dt.float32
I16 = mybir.dt.int16
P = 128
NCORES = 8
NW = 4          # gather windows
RW = 128        # table row width (elements, bf16) = 256B
AF = mybir.ActivationFunctionType
OP = mybir.AluOpType

nbf = ml_dtypes.bfloat16


# ----------------------------------------------------------------------------
# Host-side graph preprocessing
# ----------------------------------------------------------------------------

def _wcat2(tsw2, clsw2):
    w = np.zeros((128, 8), np.float32)
    w[0:64, 0:5] = tsw2
    w[64:128, 5:7] = clsw2
    return w


def _prep(inputs, tiles_per_batch=7):
    x = np.asarray(inputs["x"], np.float32)
    ei = np.asarray(inputs["edge_index"], np.int32)
    N, D_IN = x.shape
    NPC = N // NCORES
    NT = math.ceil(NPC / P)
    NPAD = NT * P
    TBL = NCORES * NPAD
    WIN = TBL // NW
    NBINS = NCORES * NT

    # self-loops are handled analytically on-device (diagonal term of the
    # segment softmax); only the regular edges go through the gather path
    src = ei[0]
    dst = ei[1]

    # ---- balanced node -> (core, tile, slot) assignment: per-round LPT —
    # each round deals one node per bin, highest degree to least-loaded ----
    deg = np.bincount(dst, minlength=N).astype(np.int64)
    order_n = np.argsort(-deg, kind="stable")
    node_bin = np.zeros(N, np.int32)
    node_slot = np.zeros(N, np.int32)
    bin_edges_load = np.zeros(NBINS, np.int64)
    for r in range(-(-N // NBINS)):
        nodes_r = order_n[r * NBINS:(r + 1) * NBINS]
        order_b = np.argsort(bin_edges_load, kind="stable")[:len(nodes_r)]
        node_bin[nodes_r] = order_b.astype(np.int32)
        node_slot[nodes_r] = r
        bin_edges_load[order_b] += deg[nodes_r]
    assert node_slot.max() < P
    node_core = node_bin // NT
    node_tile = node_bin % NT

    # per (core, tile) counts -> shared schedule (pre-relabel); the halo
    # exchange gives every core a single compact gather window
    core_of0 = node_core[dst]
    tile_of0 = node_tile[dst]
    cnt = np.zeros((NCORES, NT), np.int64)
    np.add.at(cnt, (core_of0, tile_of0), 1)
    chs = np.maximum(1, np.ceil(cnt / P).astype(np.int64).max(axis=0))  # [NT]

    # group tiles by chunk count; batches of identical structure; then
    # RELABEL tiles so the schedule visits 0,1,2,... consecutively
    keys = [int(chs[t]) for t in range(NT)]
    order_t = sorted(range(NT), key=lambda t: (keys[t], t))
    fp = np.zeros(NT, np.int64)
    fp[order_t] = np.arange(NT)
    node_tile = fp[node_tile].astype(np.int32)

    batches = []  # (t0, nb, S) over relabeled consecutive tiles
    i = 0
    while i < NT:
        j = i
        S_i = keys[order_t[i]]
        while (j < NT and keys[order_t[j]] == S_i
               and j - i < tiles_per_batch
               and (j - i + 1) * S_i <= 64):
            j += 1
        batches.append((i, j - i, S_i))
        i = j

    # chunk bookkeeping in batch order
    CH = 0
    binfo = []  # (c0, t0, nb, S)
    for t0, nb, S in batches:
        binfo.append((CH, t0, nb, S))
        CH += S * nb
    CH = int(CH)

    rowpos = node_core.astype(np.int64) * NPAD + node_tile * P + node_slot
    core_of = node_core[dst]
    tile_of = node_tile[dst]
    loc_of = node_slot[dst]
    srow = rowpos[src]
    sowner = (srow // NPAD).astype(np.int64)
    slocal = (srow % NPAD).astype(np.int64)

    # ---- halo-exchange request lists, split into two batch-coverage
    # halves so the second AllToAll overlaps the first half's compute.
    # reqH[o][c] = o's local rows that c's half-H edges need ----
    NBA = (len(binfo) + 1) // 2
    tsplit = binfo[NBA][1] if NBA < len(binfo) else NT
    halfsel = [tile_of < tsplit, tile_of >= tsplit]
    req = [[[None] * NCORES for _ in range(NCORES)] for _ in range(2)]
    for H in range(2):
        for c in range(NCORES):
            sel = (core_of == c) & halfsel[H]
            for o in range(NCORES):
                req[H][o][c] = np.unique(slocal[sel & (sowner == o)])
    PADC = [-(-max(len(req[H][o][c]) for o in range(NCORES)
                   for c in range(NCORES)) // P) * P for H in range(2)]
    assert 8 * max(PADC) <= 32767, PADC
    SCHH = [8 * PADC[H] // P for H in range(2)]

    def wrap(ids):
        a = ids.reshape(-1, 16).T.copy()
        return np.tile(a, (8, 1)).astype(np.int16)

    # per-core edge arrays (recv-window positions; batch-major chunks)
    per_core = []
    for c in range(NCORES):
        sel = core_of == c
        t_c, l_c = tile_of[sel], loc_of[sel]
        o_c, r_c = sowner[sel], slocal[sel]
        # recv position of each edge source (within its half's window)
        rpos = np.zeros(len(o_c), np.int64)
        for H in range(2):
            hs = halfsel[H][sel]
            for o in range(NCORES):
                m = hs & (o_c == o)
                rpos[m] = o * PADC[H] + np.searchsorted(req[H][o][c], r_c[m])
        sendidx = []
        for H in range(2):
            si = np.zeros(8 * PADC[H], np.int16)
            for j in range(NCORES):
                r = req[H][c][j]
                si[j * PADC[H]:j * PADC[H] + len(r)] = r
            sendidx.append(wrap(si))
        srcw = np.zeros(max(1, CH) * P, np.int16)
        dstloc = np.full((CH, P), -1.0, np.float32)
        for (c0, t0, nb, S) in binfo:
            for i_t in range(nb):
                t = t0 + i_t
                m = t_c == t
                k = int(m.sum())
                assert k <= S * P, (k, S)
                gp = c0 + i_t * S
                j = np.arange(k)
                srcw[(gp + j // P) * P + (j % P)] = rpos[m].astype(np.int16)
                dstloc[gp + j // P, j % P] = l_c[m]

        per_core.append((wrap(srcw), sendidx, dstloc.T.copy()))

    # ----- weights / constants (replicated) -----
    f32 = np.float32
    W1 = np.asarray(inputs["W1"], f32)
    as1 = np.asarray(inputs["att_src1"], f32)
    ad1 = np.asarray(inputs["att_dst1"], f32)
    W1h = W1.reshape(D_IN, 8, 8)
    wtab1 = np.concatenate(
        [W1, np.einsum("khc,hc->kh", W1h, as1), np.einsum("khc,hc->kh", W1h, ad1)], 1
    )  # [D_IN, 80]
    KA = 128 if D_IN > 128 else D_IN
    KB = D_IN - KA

    W2 = np.asarray(inputs["W2"], f32)
    as2 = np.asarray(inputs["att_src2"], f32)
    ad2 = np.asarray(inputs["att_dst2"], f32)
    W2h = W2.reshape(64, 8, 64)
    wsd2 = np.concatenate(
        [np.einsum("khc,hc->kh", W2h, as2), np.einsum("khc,hc->kh", W2h, ad2)], 1
    )  # [64, 16]
    wbig = (W2h.transpose(1, 0, 2).reshape(512, 64) / 8.0)
    w1cat_np = np.concatenate([np.asarray(inputs["ts_w1"], f32),
                               np.asarray(inputs["cls_w1"], f32)], 1)  # [64,128]
    b1cat_np = np.concatenate([np.asarray(inputs["ts_b1"], f32),
                               np.asarray(inputs["cls_b1"], f32)])     # [128]
    w1big = wbig @ w1cat_np                       # [512, 128]
    b1big = (np.asarray(inputs["b2"], f32) @ w1cat_np + b1cat_np)  # [128]
    w1big_dev = w1big.reshape(4, 128, 128).transpose(1, 0, 2).reshape(128, 512)

    consts = {
        "wtab1": wtab1.astype(nbf),
        "wsd2": wsd2.astype(nbf),
        "w1big": w1big_dev.astype(nbf),
        "b1big": b1big.reshape(P, 1),
        "wcat2": _wcat2(np.asarray(inputs["ts_w2"], f32),
                        np.asarray(inputs["cls_w2"], f32)).astype(nbf),
        "bcat2": np.concatenate([np.asarray(inputs["ts_b2"], f32),
                                 np.asarray(inputs["cls_b2"], f32),
                                 np.zeros(1, f32)]).reshape(8, 1),
        "b1r": np.tile(np.asarray(inputs["b1"], f32)[None, :], (P, 1)),
        "iota": np.tile(np.arange(P, dtype=f32)[None, :], (P, 1)).astype(nbf),
        "ident": np.eye(P, dtype=f32).astype(nbf),
        "identf": np.eye(P, dtype=f32),
        "ones": np.ones((P, 1), f32),
    }

    tst = np.asarray(inputs["timestamp_target"], np.int64)
    clt = np.asarray(inputs["node_target"], np.int64)
    msk = np.asarray(inputs["node_mask"]).astype(f32)

    in_maps = []
    pos_in_core = node_tile.astype(np.int64) * P + node_slot
    for c in range(NCORES):
        srcw, sendidx, dstloc = per_core[c]
        mine = np.nonzero(node_core == c)[0]
        pos = pos_in_core[mine]
        xT = np.zeros((D_IN, NPAD), f32)
        xT[:, pos] = x[mine].T
        valid = np.zeros(NPAD, bool)
        valid[pos] = True
        g_ts = np.zeros(NPAD, np.int64)
        g_ts[pos] = tst[mine]
        g_cl = np.zeros(NPAD, np.int64)
        g_cl[pos] = clt[mine]
        g_mk = np.zeros(NPAD, f32)
        g_mk[pos] = msk[mine]
        rows = np.arange(NPAD)
        ohts = np.zeros((NPAD, 5), f32)
        ohts[rows, g_ts] = 1.0
        ohcl = np.zeros((NPAD, 2), f32)
        ohcl[rows, g_cl] = 1.0

        def pmf(a, w):
            # [NPAD, w] -> [P, NT*w] (tiles already in schedule order)
            return a.reshape(NT, P, w).transpose(1, 0, 2).reshape(P, NT * w).copy()

        m = {
            "xT": xT.astype(nbf),
            "dstloc": dstloc.astype(nbf),
            "ohts": pmf(ohts, 5),
            "ohcl": pmf(ohcl, 2),
            "vmv": pmf(valid.astype(f32)[:, None], 1),
            "vmm": pmf((g_mk * valid)[:, None], 2 - 1),
        }
        m["srcw"] = srcw
        m["sendidxa"] = sendidx[0]
        m["sendidxb"] = sendidx[1]
        m.update(consts)
        in_maps.append(m)

    cfg = dict(N=N, D_IN=D_IN, NPC=NPC, NT=NT, NPAD=NPAD, TBL=TBL,
               CH=CH, KA=KA, KB=KB, binfo=binfo, PADC=PADC, SCHH=SCHH, NBA=NBA)
    return cfg, in_maps


# ----------------------------------------------------------------------------
# Device graph
# ----------------------------------------------------------------------------

def _build(cfg):
    import os
    STOPAT = int(os.environ.get("STOPAT", "99"))
    N, D_IN = cfg["N"], cfg["D_IN"]
    NT, NPAD, TBL = cfg["NT"], cfg["NPAD"], cfg["TBL"]
    CH = cfg["CH"]
    KA, KB = cfg["KA"], cfg["KB"]
    binfo = cfg["binfo"]
    PADC, SCHH, NBA = cfg["PADC"], cfg["SCHH"], cfg["NBA"]
    RG = [list(range(NCORES))]

    kbmax = max(S * nb for (_, _, nb, S) in binfo)
    NBMAX = max(nb for (_, _, nb, S) in binfo)

    nc = Bacc("TRN2", target_bir_lowering=False, num_devices=NCORES)

    ein = lambda name, shp, dt: nc.dram_tensor(name, shp, dt, kind="ExternalInput")
    xT_d = ein("xT", [D_IN, NPAD], BF16)
    srcw_d = ein("srcw", [P, max(1, CH) * 8], I16)
    sendidx_d = [ein("sendidxa", [P, SCHH[0] * 8], I16),
                 ein("sendidxb", [P, SCHH[1] * 8], I16)]
    dstloc_d = ein("dstloc", [P, CH], BF16)
    ohts_d = ein("ohts", [P, NT * 5], F32)
    ohcl_d = ein("ohcl", [P, NT * 2], F32)
    vmv_d = ein("vmv", [P, NT], F32)
    vmm_d = ein("vmm", [P, NT], F32)
    wtab1_d = ein("wtab1", [D_IN, 80], BF16)
    wsd2_d = ein("wsd2", [64, 16], BF16)
    w1big_d = ein("w1big", [P, 512], BF16)
    b1big_d = ein("b1big", [P, 1], F32)
    wcat2_d = ein("wcat2", [P, 8], BF16)
    bcat2_d = ein("bcat2", [8, 1], F32)
    b1r_d = ein("b1r", [P, 64], F32)
    iota_d = ein("iota", [P, P], BF16)
    identf_d = ein("identf", [P, P], F32)
    ident_d = ein("ident", [P, P], BF16)
    ones_d = ein("ones", [P, 1], F32)

    out_d = nc.dram_tensor("out", [1, 1], F32, kind="ExternalOutput")

    tbl1_loc = nc.dram_tensor("tbl1_loc", [NPAD, RW], BF16)
    tbl2_loc = nc.dram_tensor("tbl2_loc", [NPAD, RW], BF16)
    sb = [[nc.dram_tensor(f"sb{l}{h}", [8 * PADC[h], RW], BF16)
           for h in range(2)] for l in range(2)]
    rb = [[nc.dram_tensor(f"rb{l}{h}", [8 * PADC[h], RW], BF16)
           for h in range(2)] for l in range(2)]
    ar_in = nc.dram_tensor("ar_in", [1, 8], F32)
    ar_out = nc.dram_tensor("ar_out", [1, 8], F32, addr_space="Shared")

    with tile.TileContext(nc) as tc:
        with (
            tc.tile_pool(name="const", bufs=1) as cp,
            tc.tile_pool(name="sbuf", bufs=2) as sp,
            tc.tile_pool(name="stage", bufs=2) as stp,
            tc.tile_pool(name="psum", bufs=2, space="PSUM") as pp,
        ):
            # ---------------- constants to SBUF ----------------
            def ld(t, dram, shape, dt=BF16):
                s = cp.tile(shape, dt, tag=t, name=t)
                nc.sync.dma_start(out=s[: shape[0]], in_=dram[:])
                return s

            wt1a = cp.tile([KA, 80], BF16, tag="wt1a")
            nc.sync.dma_start(out=wt1a[:], in_=wtab1_d[0:KA, :])
            if KB:
                wt1b = cp.tile([max(KB, 32), 80], BF16, tag="wt1b")
                nc.sync.dma_start(out=wt1b[:KB], in_=wtab1_d[KA:D_IN, :])
            wsd2 = ld("wsd2", wsd2_d, [64, 16])
            w1big = ld("w1big", w1big_d, [P, 512])
            b1big = ld("b1big", b1big_d, [P, 1], F32)
            wcat2 = ld("wcat2", wcat2_d, [P, 8])
            bcat2 = ld("bcat2", bcat2_d, [8, 1], F32)
            b1r = ld("b1r", b1r_d, [P, 64], F32)
            iota = ld("iota", iota_d, [P, P])
            ident = ld("ident", ident_d, [P, P])
            identf = ld("identf", identf_d, [P, P], F32)
            ones = ld("ones", ones_d, [P, 1], F32)
            srcw = ld("srcw", srcw_d, [P, max(1, CH) * 8], I16)
            sendidx = [ld("sendidxa", sendidx_d[0], [P, SCHH[0] * 8], I16),
                       ld("sendidxb", sendidx_d[1], [P, SCHH[1] * 8], I16)]
            dstloc = ld("dstloc", dstloc_d, [P, CH])
            ohts = ld("ohts", ohts_d, [P, NT * 5], F32)
            ohcl = ld("ohcl", ohcl_d, [P, NT * 2], F32)
            vmv = ld("vmv", vmv_d, [P, NT], F32)
            vmm = ld("vmm", vmm_d, [P, NT], F32)

            # SBUF-resident local table caches: [feat(64)|a_src(8)|a_dst(8)]
            # per tile, written by phase A (layer 1) / fin1 (layer 2)
            tc1 = cp.tile([P, NT * 80], BF16, tag="tc1")
            tc2 = cp.tile([P, NT * 80], BF16, tag="tc2")

            acc = cp.tile([P, 4], F32, tag="acc")
            nc.vector.memset(acc[:], 0.0)

            # ---------------- phase A: layer-1 table ----------------
            WG = 7  # tiles per table-write group
            for g0 in range(0, NT, WG):
                gn = min(WG, NT - g0)
                xa = sp.tile([P, WG * P], BF16, tag="xa")
                nc.sync.dma_start(out=xa[:, 0:gn * P],
                                  in_=xT_d[0:KA, g0 * P:(g0 + gn) * P])
                if KB:
                    xb = sp.tile([max(KB, 32), WG * P], BF16, tag="xb")
                    nc.sync.dma_start(out=xb[:KB, 0:gn * P],
                                      in_=xT_d[KA:D_IN, g0 * P:(g0 + gn) * P])
                for ti in range(gn):
                    t = g0 + ti
                    pA = pp.tile([P, 512], F32, tag="agg", bufs=2)
                    if KB:
                        nc.tensor.matmul(pA[:, 0:80], lhsT=xa[:, ti * P:(ti + 1) * P],
                                         rhs=wt1a[:], start=True, stop=False)
                        nc.tensor.matmul(pA[:, 0:80], lhsT=xb[:KB, ti * P:(ti + 1) * P],
                                         rhs=wt1b[:KB], start=False, stop=True)
                    else:
                        nc.tensor.matmul(pA[:, 0:80], lhsT=xa[:, ti * P:(ti + 1) * P],
                                         rhs=wt1a[:], start=True, stop=True)
                    nc.scalar.activation(tc1[:, t * 80:(t + 1) * 80], pA[:, 0:80],
                                         AF.Copy)
                tdst = tbl1_loc[:].rearrange("(t p) w -> p t w", p=P)[:, g0:g0 + gn, 0:80]
                nc.sync.dma_start(
                    out=tdst,
                    in_=tc1[:, g0 * 80:(g0 + gn) * 80].rearrange(
                        "p (t w) -> p t w", w=80))

            def halo_exchange(tbl_loc, layer):
                # per half: gather the rows each peer requested into the send
                # buffer, then AllToAll (rank c's shard j -> rank j's shard c).
                # The second half's A2A overlaps the first half's edge compute.
                SGB = 46
                for h in range(2):
                    for p0 in range(0, SCHH[h], SGB):
                        pc = min(SGB, SCHH[h] - p0)
                        gs = sp.tile([P, SGB * RW], BF16, tag="sgb", name="sgb")
                        nc.gpsimd.dma_gather(
                            out_ap=gs[:, 0:pc * RW].rearrange(
                                "p (c e) -> p c e", e=RW),
                            in_ap=tbl_loc[:],
                            idxs_ap=sendidx[h][:, p0 * 8:(p0 + pc) * 8],
                            num_idxs=pc * P, num_idxs_reg=pc * P, elem_size=RW,
                            single_packet=False)
                        nc.sync.dma_start(
                            out=sb[layer][h][:].rearrange(
                                "(c p) e -> p c e", p=P)[:, p0:p0 + pc, :],
                            in_=gs[:, 0:pc * RW].rearrange(
                                "p (c e) -> p c e", e=RW))
                    nc.gpsimd.collective_compute(
                        "AllToAll", OP.bypass, ins=[sb[layer][h][:]],
                        outs=[rb[layer][h][:]], replica_groups=RG,
                    )

            if STOPAT >= 1:
                halo_exchange(tbl1_loc, 0)

            # ---------------- edge phases ----------------
            def edge_layer(layer, tcache, fin_group):
                WM = 72 if layer == 1 else 520
                FW = 64 if layer == 1 else 512
                for bi, (c0, t0, nb, S) in enumerate(binfo):
                    kb = nb * S
                    gm = sp.tile([P, kbmax * RW], BF16, tag="gm")
                    nc.gpsimd.dma_gather(
                        out_ap=gm[:, 0:kb * RW].rearrange("p (c e) -> p c e", e=RW),
                        in_ap=rb[layer - 1][0 if bi < NBA else 1][:],
                        idxs_ap=srcw[:, c0 * 8:(c0 + kb) * 8],
                        num_idxs=kb * P, num_idxs_reg=kb * P, elem_size=RW,
                        single_packet=False)

                    # one-hot [edge, slot] per chunk (window-major dstloc)
                    oh = sp.tile([P, kbmax * P], BF16, tag="oh")
                    nc.vector.tensor_tensor(
                        out=oh[:, 0:kb * P].rearrange("p (c e) -> p c e", e=P),
                        in0=dstloc[:, c0:c0 + kb].unsqueeze(2).to_broadcast(
                            [P, kb, P]),
                        in1=iota[:].unsqueeze(1).to_broadcast([P, kb, P]),
                        op=OP.is_equal,
                    )

                    # transposed one-hots: PE transposes into PSUM slabs,
                    # batched PSUM->SBUF copies, then per-chunk 8-col matmuls
                    # against the local tile's a_dst columns
                    ohT = sp.tile([P, kbmax * P], BF16, tag="ohT")
                    for h0 in range(0, kb, 7):
                        hn = min(7, kb - h0)
                        tpb = pp.tile([P, 7 * P], BF16, tag="tpbB", bufs=2)
                        for i in range(hn):
                            nc.tensor.transpose(tpb[:, i * P:(i + 1) * P],
                                                oh[:, (h0 + i) * P:(h0 + i + 1) * P],
                                                ident[:])
                        nc.scalar.activation(ohT[:, h0 * P:(h0 + hn) * P],
                                             tpb[:, 0:hn * P], AF.Copy)
                    adpe = pp.tile([P, kbmax * 8], F32, tag="adpe", bufs=1)
                    for i_t in range(nb):
                        for q in range(S):
                            jj = i_t * S + q
                            nc.tensor.matmul(
                                adpe[:, jj * 8:(jj + 1) * 8],
                                lhsT=ohT[:, jj * P:(jj + 1) * P],
                                rhs=tcache[:, (t0 + i_t) * 80 + 72:
                                           (t0 + i_t) * 80 + 80],
                                start=True, stop=True)

                    # alpha / leaky relu / exp / weighted messages: one op per
                    # batch (window-major layout is contiguous)
                    alpha = sp.tile([P, kbmax * 8], F32, tag="alpha", bufs=1)
                    msg = sp.tile([P, kbmax * WM], BF16, tag="msg")
                    g4 = gm[:, 0:kb * RW].rearrange("p (c e) -> p c e", e=RW)
                    ms3 = msg[:, 0:kb * WM].rearrange("p (c e) -> p c e", e=WM)
                    nc.vector.tensor_tensor(
                        out=alpha[:, 0:kb * 8].rearrange("p (c e) -> p c e", e=8),
                        in0=g4[:, :, 64:72],
                        in1=adpe[:, 0:kb * 8].rearrange("p (c e) -> p c e", e=8),
                        op=OP.add)
                    nc.vector.scalar_tensor_tensor(
                        out=alpha[:, 0:kb * 8],
                        in0=alpha[:, 0:kb * 8], scalar=0.2,
                        in1=alpha[:, 0:kb * 8], op0=OP.mult, op1=OP.max)
                    # exp straight into the msg tail (denominator columns)
                    nc.scalar.activation(
                        ms3[:, :, WM - 8:WM],
                        alpha[:, 0:kb * 8].rearrange("p (c e) -> p c e", e=8),
                        AF.Exp)
                    if layer == 1:
                        nc.vector.tensor_tensor(
                            out=ms3[:, :, 0:64].rearrange("p c (h z) -> p c h z", h=8),
                            in0=g4[:, :, 0:64].rearrange("p c (h z) -> p c h z", h=8),
                            in1=ms3[:, :, 64:72].unsqueeze(3).to_broadcast(
                                [P, kb, 8, 8]),
                            op=OP.mult,
                        )
                    else:
                        nc.vector.tensor_tensor(
                            out=ms3[:, :, 0:512].rearrange("p c (h z) -> p c h z", h=8),
                            in0=g4[:, :, 0:64].unsqueeze(2).to_broadcast(
                                [P, kb, 8, 64]),
                            in1=ms3[:, :, 512:520].unsqueeze(3).to_broadcast(
                                [P, kb, 8, 64]),
                            op=OP.mult,
                        )

                    # per-tile aggregation + PSUM->SBUF copy into group slabs
                    FWW = FW + (8 if layer == 1 else 0)
                    pzs = sp.tile([P, NBMAX * FWW], F32 if layer == 1 else BF16,
                                  tag=f"pzs{layer}", bufs=1,
                                  name="pzs")
                    pds = (sp.tile([P, NBMAX * 8], F32, tag="pds", bufs=1, name="pds")
                           if layer == 2 else None)
                    pdp = (pp.tile([P, kbmax * 8], F32, tag="adpe", bufs=1,
                                   name="pdp")
                           if layer == 2 else None)
                    for i_t in range(nb):
                        pz = pp.tile([P, 512], F32, tag="agg", bufs=2, name="pz")
                        pd = (pdp[:, i_t * 8:(i_t + 1) * 8]
                              if layer == 2 else None)
                        for q in range(S):
                            jj = i_t * S + q
                            ohj = oh[:, jj * P:(jj + 1) * P]
                            mj = msg[:, jj * WM:(jj + 1) * WM]
                            st, fi = (q == 0), (q == S - 1)
                            nc.tensor.matmul(pz[:, 0:FWW], lhsT=ohj,
                                             rhs=mj[:, 0:FWW],
                                             start=st, stop=fi)
                            if layer == 2:
                                nc.tensor.matmul(pd[:], lhsT=ohj,
                                                 rhs=mj[:, 512:520],
                                                 start=st, stop=fi)
                        nc.scalar.activation(pzs[:, i_t * FWW:(i_t + 1) * FWW],
                                             pz[:, 0:FWW], AF.Copy)
                        if layer == 2:
                            nc.vector.tensor_copy(pds[:, i_t * 8:(i_t + 1) * 8],
                                                  pd[:])
                    fin_group(t0, nb, pzs, pds)

            # ---------------- group finalizers ----------------
            def selfloop_ea_grp(tcache, t0, nb):
                # ea of each node's own self-loop: exp(lrelu(a_src + a_dst))
                tg = tcache[:, t0 * 80:(t0 + nb) * 80].rearrange(
                    "p (t w) -> p t w", w=80)
                asum = sp.tile([P, NBMAX * 8], F32, tag="asum", bufs=1)
                nc.vector.tensor_tensor(
                    out=asum[:, 0:nb * 8].rearrange("p (t e) -> p t e", e=8),
                    in0=tg[:, :, 64:72], in1=tg[:, :, 72:80], op=OP.add)
                lrs = sp.tile([P, NBMAX * 8], F32, tag="lrs", bufs=1)
                nc.vector.scalar_tensor_tensor(
                    out=lrs[:, 0:nb * 8], in0=asum[:, 0:nb * 8], scalar=0.2,
                    in1=asum[:, 0:nb * 8], op0=OP.mult, op1=OP.max)
                eas = sp.tile([P, NBMAX * 8], F32, tag="eas", bufs=1)
                nc.scalar.activation(eas[:, 0:nb * 8], lrs[:, 0:nb * 8], AF.Exp)
                return eas

            t2_state = {"n": 0}

            def fin1_group(t0, nb, pzs, pds):
                # pzs: [P, nb*72] = [num(64) | denom(8)] per tile
                pz3 = pzs[:, 0:nb * 72].rearrange("p (t e) -> p t e", e=72)
                tg = tc1[:, t0 * 80:(t0 + nb) * 80].rearrange(
                    "p (t w) -> p t w", w=80)
                eas = selfloop_ea_grp(tc1, t0, nb)
                ea3 = eas[:, 0:nb * 8].rearrange("p (t e) -> p t e", e=8)
                rin = sp.tile([P, NBMAX * 8], F32, tag="rin", bufs=1)
                nc.vector.scalar_tensor_tensor(
                    out=rin[:, 0:nb * 8].rearrange("p (t e) -> p t e", e=8),
                    in0=pz3[:, :, 64:72], scalar=1e-16, in1=ea3,
                    op0=OP.add, op1=OP.add)
                rcp = sp.tile([P, NBMAX * 8], F32, tag="rcp", bufs=1)
                nc.vector.reciprocal(rcp[:, 0:nb * 8], rin[:, 0:nb * 8])
                num = sp.tile([P, NBMAX * 64], F32, tag="num", bufs=1)
                nc.vector.tensor_tensor(
                    out=num[:, 0:nb * 64].rearrange("p (t h c) -> p t h c", h=8, c=8),
                    in0=tg[:, :, 0:64].rearrange("p t (h c) -> p t h c", h=8),
                    in1=ea3.unsqueeze(3).to_broadcast([P, nb, 8, 8]),
                    op=OP.mult,
                )
                nc.vector.tensor_tensor(
                    out=num[:, 0:nb * 64].rearrange("p (t e) -> p t e", e=64),
                    in0=num[:, 0:nb * 64].rearrange("p (t e) -> p t e", e=64),
                    in1=pz3[:, :, 0:64], op=OP.add)
                h1f = sp.tile([P, NBMAX * 64], F32, tag="h1f", bufs=1)
                nc.vector.tensor_tensor(
                    out=h1f[:, 0:nb * 64].rearrange("p (t h c) -> p t h c", h=8, c=8),
                    in0=num[:, 0:nb * 64].rearrange("p (t h c) -> p t h c", h=8, c=8),
                    in1=rcp[:, 0:nb * 8].rearrange("p (t e) -> p t e", e=8)
                        .unsqueeze(3).to_broadcast([P, nb, 8, 8]),
                    op=OP.mult,
                )
                # layer-2 table rows: feat = h1f + b1, attn via wsd2
                trow = stp.tile([P, NBMAX * RW], BF16, tag="tbl2_w", name="tbl2w")
                nc.vector.tensor_tensor(
                    out=trow[:, 0:nb * RW].rearrange(
                        "p (t e) -> p t e", e=RW)[:, :, 0:64],
                    in0=h1f[:, 0:nb * 64].rearrange("p (t e) -> p t e", e=64),
                    in1=b1r[:].unsqueeze(1).to_broadcast([P, nb, 64]),
                    op=OP.add)
                # transposes of the nb feature blocks + one batched copy
                tpb = pp.tile([P, 7 * P], BF16, tag="tpbB", bufs=2)
                for i_t in range(nb):
                    nc.tensor.transpose(
                        tpb[0:64, i_t * P:(i_t + 1) * P],
                        trow[:, i_t * RW:i_t * RW + 64], ident[:])
                h1T = sp.tile([64, 7 * P], BF16, tag="h1T", bufs=1)
                nc.scalar.activation(h1T[:, 0:nb * P], tpb[0:64, 0:nb * P], AF.Copy)
                pf = pp.tile([P, NBMAX * 64], F32, tag="hp", bufs=1)
                for i_t in range(nb):
                    nc.tensor.matmul(pf[:, i_t * 16:(i_t + 1) * 16],
                                     lhsT=h1T[:, i_t * P:(i_t + 1) * P],
                                     rhs=wsd2[:], start=True, stop=True)
                nc.scalar.activation(
                    trow[:, 0:nb * RW].rearrange("p (t e) -> p t e", e=RW)[:, :, 64:80],
                    pf[:, 0:nb * 16].rearrange("p (t e) -> p t e", e=16), AF.Copy)
                nc.vector.tensor_copy(
                    tc2[:, t0 * 80:(t0 + nb) * 80].rearrange(
                        "p (t e) -> p t e", e=80),
                    trow[:, 0:nb * RW].rearrange("p (t e) -> p t e", e=RW)[:, :, 0:80])
                tdst = tbl2_loc[:].rearrange("(t p) w -> p t w", p=P)[
                    :, t0:t0 + nb, 0:80]
                nc.sync.dma_start(
                    out=tdst,
                    in_=trow[:, 0:nb * RW].rearrange(
                        "p (t e) -> p t e", e=RW)[:, :, 0:80])
                t2_state["n"] += nb

            if STOPAT >= 2:
                edge_layer(1, tc1, fin1_group)

            if STOPAT >= 3:
                halo_exchange(tbl2_loc, 1)

            # ---------------- layer-2 finalize: h2, MLPs, CE ----------------
            ceall_ts = cp.tile([P, NT], F32, tag="cets")
            ceall_cl = cp.tile([P, NT], F32, tag="cecl")
            sum2 = cp.tile([P, 2 * NT], F32, tag="sum2")   # [ts | cl] exp-sums
            pk2 = cp.tile([P, 2 * NT], F32, tag="pk2")     # picked logits

            def fin2_group(gt0, gnb, gpzs, gpds):
              for o0 in range(0, gnb, 4):
                nb = min(4, gnb - o0)
                t0 = gt0 + o0
                pzs = gpzs[:, o0 * 512:(o0 + nb) * 512]
                pds = gpds[:, o0 * 8:(o0 + nb) * 8]
                # pzs: [P, nb*512] per-head numerators; pds: [P, nb*8] denoms
                tg = tc2[:, t0 * 80:(t0 + nb) * 80].rearrange(
                    "p (t w) -> p t w", w=80)
                eas = selfloop_ea_grp(tc2, t0, nb)
                ea3 = eas[:, 0:nb * 8].rearrange("p (t e) -> p t e", e=8)
                rin = sp.tile([P, NBMAX * 8], F32, tag="rin", bufs=1)
                nc.vector.scalar_tensor_tensor(
                    out=rin[:, 0:nb * 8].rearrange("p (t e) -> p t e", e=8),
                    in0=pds[:, 0:nb * 8].rearrange("p (t e) -> p t e", e=8),
                    scalar=1e-16, in1=ea3, op0=OP.add, op1=OP.add)
                rcp = sp.tile([P, NBMAX * 8], F32, tag="rcp", bufs=1)
                nc.vector.reciprocal(rcp[:, 0:nb * 8], rin[:, 0:nb * 8])
                num = sp.tile([P, NBMAX * 512], F32, tag="num2", bufs=1)
                nc.vector.tensor_tensor(
                    out=num[:, 0:nb * 512].rearrange(
                        "p (t h c) -> p t h c", h=8, c=64),
                    in0=tg[:, :, 0:64].unsqueeze(2).to_broadcast([P, nb, 8, 64]),
                    in1=ea3.unsqueeze(3).to_broadcast([P, nb, 8, 64]),
                    op=OP.mult,
                )
                nc.vector.tensor_tensor(
                    out=num[:, 0:nb * 512],
                    in0=num[:, 0:nb * 512], in1=pzs[:, 0:nb * 512], op=OP.add)
                zn = sp.tile([P, NBMAX * 512], BF16, tag="zn", bufs=1)
                nc.vector.tensor_tensor(
                    out=zn[:, 0:nb * 512].rearrange(
                        "p (t h c) -> p t h c", h=8, c=64),
                    in0=num[:, 0:nb * 512].rearrange(
                        "p (t h c) -> p t h c", h=8, c=64),
                    in1=rcp[:, 0:nb * 8].rearrange("p (t e) -> p t e", e=8)
                        .unsqueeze(3).to_broadcast([P, nb, 8, 64]),
                    op=OP.mult,
                )
                # k-major transposes of zn; composed (wbig @ mlp-w1) matmul
                nblk = nb * 4
                zT = sp.tile([P, NBMAX * 4 * P], BF16, tag="zT", bufs=1)
                pos = 0
                tpb = None
                for k in range(4):
                    for i_t in range(nb):
                        if pos % 7 == 0:
                            if pos:
                                nc.scalar.activation(
                                    zT[:, (pos - 7) * P:pos * P],
                                    tpb[:, 0:7 * P], AF.Copy)
                            tpb = pp.tile([P, 7 * P], BF16, tag="tpbB", bufs=2)
                        nc.tensor.transpose(
                            tpb[:, (pos % 7) * P:(pos % 7 + 1) * P],
                            zn[:, (i_t * 4 + k) * P:(i_t * 4 + k + 1) * P],
                            ident[:])
                        pos += 1
                rem = pos % 7 or 7
                nc.scalar.activation(zT[:, (pos - rem) * P:pos * P],
                                     tpb[:, 0:rem * P], AF.Copy)
                pw = nb * P
                pa = pp.tile([P, 7 * P], F32, tag="tp", bufs=1, name="pa")
                for k in range(4):
                    nc.tensor.matmul(pa[:, 0:pw], lhsT=w1big[:, k * P:(k + 1) * P],
                                     rhs=zT[:, k * nb * P:(k * nb + nb) * P],
                                     start=(k == 0), stop=(k == 3))
                h12T = sp.tile([P, 7 * P], BF16, tag="h12T", bufs=1)
                nc.scalar.activation(h12T[:, 0:pw], pa[:, 0:pw], AF.Relu,
                                     bias=b1big[:, 0:1])
                lgx = pp.tile([P, 7 * P], F32, tag="tp", bufs=1, name="lg")
                lg = lgx[0:8]
                nc.tensor.matmul(lg[0:8, 0:pw], lhsT=wcat2[:],
                                 rhs=h12T[:, 0:pw], start=True, stop=True)
                lgsm = sp.tile([8, 7 * P], F32, tag="lgsm", bufs=1)
                nc.scalar.activation(lgsm[0:8, 0:pw], lg[0:8, 0:pw],
                                     AF.Identity, bias=bcat2[0:8, 0:1])
                ptlx = pp.tile([P, NBMAX * 64], F32, tag="hp", bufs=1,
                               name="ptl")
                ptl = ptlx[:, 0:NBMAX * 8]
                for i_t in range(nb):
                    nc.tensor.matmul(ptl[:, i_t * 8:(i_t + 1) * 8],
                                     lhsT=lgsm[0:8, i_t * P:(i_t + 1) * P],
                                     rhs=identf[0:8, 0:8], is_transpose=True,
                                     start=True, stop=True)
                # batched CE over the group's tiles
                tl3 = ptl[:, 0:nb * 8].rearrange("p (t e) -> p t e", e=8)
                ex_ts = sp.tile([P, NBMAX * 5], F32, tag="exts", bufs=1)
                ex_cl = sp.tile([P, NBMAX * 2], F32, tag="excl", bufs=1)
                nc.scalar.activation(
                    ex_ts[:, 0:nb * 5].rearrange("p (t e) -> p t e", e=5),
                    tl3[:, :, 0:5], AF.Exp)
                nc.scalar.activation(
                    ex_cl[:, 0:nb * 2].rearrange("p (t e) -> p t e", e=2),
                    tl3[:, :, 5:7], AF.Exp)
                nc.vector.reduce_sum(
                    sum2[:, t0:t0 + nb].rearrange("p (t e) -> p t e", e=1),
                    ex_ts[:, 0:nb * 5].rearrange("p (t e) -> p t e", e=5),
                    axis=mybir.AxisListType.X)
                nc.vector.reduce_sum(
                    sum2[:, NT + t0:NT + t0 + nb].rearrange("p (t e) -> p t e", e=1),
                    ex_cl[:, 0:nb * 2].rearrange("p (t e) -> p t e", e=2),
                    axis=mybir.AxisListType.X)
                pk_ts = sp.tile([P, NBMAX * 5], F32, tag="pkts", bufs=1)
                pk_cl = sp.tile([P, NBMAX * 2], F32, tag="pkcl", bufs=1)
                nc.vector.tensor_tensor(
                    out=pk_ts[:, 0:nb * 5].rearrange("p (t e) -> p t e", e=5),
                    in0=tl3[:, :, 0:5],
                    in1=ohts[:, t0 * 5:(t0 + nb) * 5].rearrange(
                        "p (t e) -> p t e", e=5), op=OP.mult)
                nc.vector.tensor_tensor(
                    out=pk_cl[:, 0:nb * 2].rearrange("p (t e) -> p t e", e=2),
                    in0=tl3[:, :, 5:7],
                    in1=ohcl[:, t0 * 2:(t0 + nb) * 2].rearrange(
                        "p (t e) -> p t e", e=2), op=OP.mult)
                nc.vector.reduce_sum(
                    pk2[:, t0:t0 + nb].rearrange("p (t e) -> p t e", e=1),
                    pk_ts[:, 0:nb * 5].rearrange("p (t e) -> p t e", e=5),
                    axis=mybir.AxisListType.X)
                nc.vector.reduce_sum(
                    pk2[:, NT + t0:NT + t0 + nb].rearrange("p (t e) -> p t e", e=1),
                    pk_cl[:, 0:nb * 2].rearrange("p (t e) -> p t e", e=2),
                    axis=mybir.AxisListType.X)

            if STOPAT >= 4:
                edge_layer(2, tc2, fin2_group)
                lse2 = cp.tile([P, 2 * NT], F32, tag="lse2")
                nc.scalar.activation(lse2[:], sum2[:], AF.Ln)
                nc.vector.tensor_sub(lse2[:], lse2[:], pk2[:])
                nc.vector.tensor_tensor(out=ceall_ts[:], in0=lse2[:, 0:NT],
                                        in1=vmv[:], op=OP.mult)
                nc.vector.tensor_tensor(out=ceall_cl[:], in0=lse2[:, NT:2 * NT],
                                        in1=vmm[:], op=OP.mult)
                nc.vector.reduce_sum(acc[:, 0:1], ceall_ts[:],
                                     axis=mybir.AxisListType.X)
                nc.vector.reduce_sum(acc[:, 1:2], ceall_cl[:],
                                     axis=mybir.AxisListType.X)
                nc.vector.reduce_sum(acc[:, 2:3], vmm[:],
                                     axis=mybir.AxisListType.X)

            # ---------------- final reduction ----------------
            pfinx = pp.tile([P, 7 * P], F32, tag="tp", bufs=1)
            pfin = pfinx[0:1, 0:8]
            nc.tensor.matmul(pfin[0:1, 0:3], lhsT=ones[:], rhs=acc[:, 0:3],
                             start=True, stop=True)
            fin_sb = cp.tile([1, 8], F32, tag="fin")
            nc.vector.memset(fin_sb[:], 0.0)
            nc.scalar.activation(fin_sb[0:1, 0:3], pfin[0:1, 0:3], AF.Copy)
            nc.sync.dma_start(out=ar_in[:], in_=fin_sb[:])
            nc.gpsimd.collective_compute(
                "AllReduce", OP.add, ins=[ar_in[:]], outs=[ar_out[:]],
                replica_groups=RG,
            )
            tot = cp.tile([1, 8], F32, tag="tot")
            nc.sync.dma_start(out=tot[:], in_=ar_out[:])
            rcpm = cp.tile([1, 1], F32, tag="rcpm")
            nc.vector.reciprocal(rcpm[:], tot[:, 2:3])
            lcl = cp.tile([1, 1], F32, tag="lcl")
            nc.vector.tensor_tensor(out=lcl[:], in0=tot[:, 1:2], in1=rcpm[:], op=OP.mult)
            lts = cp.tile([1, 1], F32, tag="lts")
            nc.vector.tensor_scalar_mul(lts[:], tot[:, 0:1], 1.0 / N)
            res = cp.tile([1, 1], F32, tag="res")
            nc.vector.tensor_add(res[:], lcl[:], lts[:])
            nc.sync.dma_start(out=out_d[:], in_=res[:])

    nc.compile()
    return nc


# ----------------------------------------------------------------------------
# Entry points
# ----------------------------------------------------------------------------

def _run(inputs, trace=False):
    cfg, in_maps = _prep(inputs)
    nc = _build(cfg)
    try:
        r = run_bass_kernel_spmd(nc, in_maps, core_ids=list(range(NCORES)), trace=trace)
    except ModuleNotFoundError:
        r = run_bass_kernel_spmd(nc, in_maps, core_ids=list(range(NCORES)), trace=False)
    out = np.asarray(r.results[0]["out"], np.float32).reshape(())
    return out, r


def kernel(**inputs):
    out, _ = _run(inputs, trace=False)
    return out


def _build_null(cfg):
    """Same I/O signature, trivial compute — for dispatch/transfer baseline."""
    N, D_IN = cfg["N"], cfg["D_IN"]
    NT, NPAD, TBL = cfg["NT"], cfg["NPAD"], cfg["TBL"]
    CH = cfg["CH"]
    nc = Bacc("TRN2", target_bir_lowering=False, num_devices=NCORES)
    ein = lambda name, shp, dt: nc.dram_tensor(name, shp, dt, kind="ExternalInput")
    xT_d = ein("xT", [D_IN, NPAD], BF16)
    ein("srcw", [P, max(1, CH) * 8], I16)
    ein("sendidxa", [P, cfg["SCHH"][0] * 8], I16)
    ein("sendidxb", [P, cfg["SCHH"][1] * 8], I16)
    ein("dstloc", [P, CH], BF16)
    ein("ohts", [P, NT * 5], F32)
    ein("ohcl", [P, NT * 2], F32)
    ein("vmv", [P, NT], F32)
    ein("vmm", [P, NT], F32)
    ein("wtab1", [D_IN, 80], BF16)
    ein("wsd2", [64, 16], BF16)
    ein("w1big", [P, 512], BF16)
    ein("b1big", [P, 1], F32)
    ein("wcat2", [P, 8], BF16)
    ein("bcat2", [8, 1], F32)
    ein("b1r", [P, 64], F32)
    ein("iota", [P, P], BF16)
    identf_d = ein("identf", [P, P], F32)
    ein("ident", [P, P], BF16)
    ein("ones", [P, 1], F32)
    out_d = nc.dram_tensor("out", [1, 1], F32, kind="ExternalOutput")
    with tile.TileContext(nc) as tc:
        with tc.tile_pool(name="sp", bufs=1) as sp:
            t = sp.tile([1, 1], F32, tag="t")
            nc.sync.dma_start(out=t[:], in_=identf_d[0:1, 0:1])
            nc.sync.dma_start(out=out_d[:], in_=t[:])
    nc.compile()
    return nc


# revision 27
# speedup vs baseline: 1.0089x; 1.0089x over previous
"""Distributed Bass kernel for nn_AdaGNN (2-layer GAT + MLP heads + CE losses).

Strategy (8 NeuronCores, SPMD):
  - Nodes assigned to 8 cores x NT tiles of 128 by a load-balancing packer
    (equal edge counts per tile). Output is permutation invariant. Tiles are
    renumbered into schedule (finalize) order so every per-tile cache slice
    is contiguous and table rows are written sequentially.
  - Per layer: dense per-node transform producing a 128-wide (256B) gather-table
    row [feat(64) | a_src(8) | a_dst(8) | pad] bf16 -> AllGather -> per-edge
    dma_gather (int16 indices; 4 source windows of TBL/4 rows each) ->
    per-edge a_dst via batched PE transposes of the one-hots + 8-col matmuls
    -> segment softmax via exp (value ranges are small; max-subtraction
    unnecessary) -> weighted segment-sum via one-hot matmuls on TensorE ->
    batched per-group normalize.
  - Edge chunks of 128 are keyed (tile, window, q) with a per-tile/window
    chunk schedule shared by all cores (SPMD-static); tiles are grouped into
    batches of identical schedule vectors; all per-edge tensors are laid out
    window-major so per-batch vector ops are single instructions.
  - Layer 2 aggregates per-head-weighted 64-dim inputs (512-wide messages) and
    applies the reshuffled W2 (mean over heads folded in) after aggregation.
  - Finalize runs per batch: per-tile PSUM->SBUF copy, then group-batched
    softmax-normalization / layer-2 table build / MLP heads / CE.
  - Partial CE sums AllReduced; final scalar computed on device.
"""

import math
import numpy as np
import ml_dtypes

import concourse.bass as bass
import concourse.tile as tile
from concourse import mybir
from concourse.bacc import Bacc
from concourse.bass_utils import run_bass_kernel_spmd

BF16 = mybir.dt.bfloat16
F32 = mybir.dt.float32
I16 = mybir.dt.int16
P = 128
NCORES = 8
NW = 4          # gather windows
RW = 128        # table row width (elements, bf16) = 256B
AF = mybir.ActivationFunctionType
OP = mybir.AluOpType

nbf = ml_dtypes.bfloat16


# ----------------------------------------------------------------------------
# Host-side graph preprocessing
# ----------------------------------------------------------------------------

def _wcat2(tsw2, clsw2):
    w = np.zeros((128, 8), np.float32)
    w[0:64, 0:5] = tsw2
    w[64:128, 5:7] = clsw2
    return w


def _prep(inputs, tiles_per_batch=7):
    x = np.asarray(inputs["x"], np.float32)
    ei = np.asarray(inputs["edge_index"], np.int32)
    N, D_IN = x.shape
    NPC = N // NCORES
    NT = math.ceil(NPC / P)
    NPAD = NT * P
    TBL = NCORES * NPAD
    WIN = TBL // NW
    NBINS = NCORES * NT

    # self-loops are handled analytically on-device (diagonal term of the
    # segment softmax); only the regular edges go through the gather path
    src = ei[0]
    dst = ei[1]

    # ---- balanced node -> (core, tile, slot) assignment: per-round LPT —
    # each round deals one node per bin, highest degree to least-loaded ----
    deg = np.bincount(dst, minlength=N).astype(np.int64)
    order_n = np.argsort(-deg, kind="stable")
    node_bin = np.zeros(N, np.int32)
    node_slot = np.zeros(N, np.int32)
    bin_edges_load = np.zeros(NBINS, np.int64)
    for r in range(-(-N // NBINS)):
        nodes_r = order_n[r * NBINS:(r + 1) * NBINS]
        order_b = np.argsort(bin_edges_load, kind="stable")[:len(nodes_r)]
        node_bin[nodes_r] = order_b.astype(np.int32)
        node_slot[nodes_r] = r
        bin_edges_load[order_b] += deg[nodes_r]
    assert node_slot.max() < P
    node_core = node_bin // NT
    node_tile = node_bin % NT

    # per (core, tile) counts -> shared schedule (pre-relabel); the halo
    # exchange gives every core a single compact gather window
    core_of0 = node_core[dst]
    tile_of0 = node_tile[dst]
    cnt = np.zeros((NCORES, NT), np.int64)
    np.add.at(cnt, (core_of0, tile_of0), 1)
    chs = np.maximum(1, np.ceil(cnt / P).astype(np.int64).max(axis=0))  # [NT]

    # group tiles by chunk count; batches of identical structure; then
    # RELABEL tiles so the schedule visits 0,1,2,... consecutively
    keys = [int(chs[t]) for t in range(NT)]
    order_t = sorted(range(NT), key=lambda t: (keys[t], t))
    fp = np.zeros(NT, np.int64)
    fp[order_t] = np.arange(NT)
    node_tile = fp[node_tile].astype(np.int32)

    batches = []  # (t0, nb, S) over relabeled consecutive tiles
    i = 0
    while i < NT:
        j = i
        S_i = keys[order_t[i]]
        while (j < NT and keys[order_t[j]] == S_i
               and j - i < tiles_per_batch
               and (j - i + 1) * S_i <= 64):
            j += 1
        batches.append((i, j - i, S_i))
        i = j

    # chunk bookkeeping in batch order
    CH = 0
    binfo = []  # (c0, t0, nb, S)
    for t0, nb, S in batches:
        binfo.append((CH, t0, nb, S))
        CH += S * nb
    CH = int(CH)

    rowpos = node_core.astype(np.int64) * NPAD + node_tile * P + node_slot
    core_of = node_core[dst]
    tile_of = node_tile[dst]
    loc_of = node_slot[dst]
    srow = rowpos[src]
    sowner = (srow // NPAD).astype(np.int64)
    slocal = (srow % NPAD).astype(np.int64)

    # ---- halo-exchange request lists, split into two batch-coverage
    # halves so the second AllToAll overlaps the first half's compute.
    # reqH[o][c] = o's local rows that c's half-H edges need ----
    NBA = (len(binfo) + 1) // 2
    tsplit = binfo[NBA][1] if NBA < len(binfo) else NT
    halfsel = [tile_of < tsplit, tile_of >= tsplit]
    req = [[[None] * NCORES for _ in range(NCORES)] for _ in range(2)]
    for H in range(2):
        for c in range(NCORES):
            sel = (core_of == c) & halfsel[H]
            for o in range(NCORES):
                req[H][o][c] = np.unique(slocal[sel & (sowner == o)])
    PADC = [-(-max(len(req[H][o][c]) for o in range(NCORES)
                   for c in range(NCORES)) // P) * P for H in range(2)]
    assert 8 * max(PADC) <= 32767, PADC
    SCHH = [8 * PADC[H] // P for H in range(2)]

    def wrap(ids):
        a = ids.reshape(-1, 16).T.copy()
        return np.tile(a, (8, 1)).astype(np.int16)

    # per-core edge arrays (recv-window positions; batch-major chunks)
    per_core = []
    for c in range(NCORES):
        sel = core_of == c
        t_c, l_c = tile_of[sel], loc_of[sel]
        o_c, r_c = sowner[sel], slocal[sel]
        # recv position of each edge source (within its half's window)
        rpos = np.zeros(len(o_c), np.int64)
        for H in range(2):
            hs = halfsel[H][sel]
            for o in range(NCORES):
                m = hs & (o_c == o)
                rpos[m] = o * PADC[H] + np.searchsorted(req[H][o][c], r_c[m])
        sendidx = []
        for H in range(2):
            si = np.zeros(8 * PADC[H], np.int16)
            for j in range(NCORES):
                r = req[H][c][j]
                si[j * PADC[H]:j * PADC[H] + len(r)] = r
            sendidx.append(wrap(si))
        srcw = np.zeros(max(1, CH) * P, np.int16)
        dstloc = np.full((CH, P), -1.0, np.float32)
        for (c0, t0, nb, S) in binfo:
            for i_t in range(nb):
                t = t0 + i_t
                m = t_c == t
                k = int(m.sum())
                assert k <= S * P, (k, S)
                gp = c0 + i_t * S
                j = np.arange(k)
                srcw[(gp + j // P) * P + (j % P)] = rpos[m].astype(np.int16)
                dstloc[gp + j // P, j % P] = l_c[m]

        per_core.append((wrap(srcw), sendidx, dstloc.T.copy()))

    # ----- weights / constants (replicated) -----
    f32 = np.float32
    W1 = np.asarray(inputs["W1"], f32)
    as1 = np.asarray(inputs["att_src1"], f32)
    ad1 = np.asarray(inputs["att_dst1"], f32)
    W1h = W1.reshape(D_IN, 8, 8)
    wtab1 = np.concatenate(
        [W1, np.einsum("khc,hc->kh", W1h, as1), np.einsum("khc,hc->kh", W1h, ad1)], 1
    )  # [D_IN, 80]
    KA = 128 if D_IN > 128 else D_IN
    KB = D_IN - KA

    W2 = np.asarray(inputs["W2"], f32)
    as2 = np.asarray(inputs["att_src2"], f32)
    ad2 = np.asarray(inputs["att_dst2"], f32)
    W2h = W2.reshape(64, 8, 64)
    wsd2 = np.concatenate(
        [np.einsum("khc,hc->kh", W2h, as2), np.einsum("khc,hc->kh", W2h, ad2)], 1
    )  # [64, 16]
    wbig = (W2h.transpose(1, 0, 2).reshape(512, 64) / 8.0)
    w1cat_np = np.concatenate([np.asarray(inputs["ts_w1"], f32),
                               np.asarray(inputs["cls_w1"], f32)], 1)  # [64,128]
    b1cat_np = np.concatenate([np.asarray(inputs["ts_b1"], f32),
                               np.asarray(inputs["cls_b1"], f32)])     # [128]
    w1big = wbig @ w1cat_np                       # [512, 128]
    b1big = (np.asarray(inputs["b2"], f32) @ w1cat_np + b1cat_np)  # [128]
    w1big_dev = w1big.reshape(4, 128, 128).transpose(1, 0, 2).reshape(128, 512)

    consts = {
        "wtab1": wtab1.astype(nbf),
        "wsd2": wsd2.astype(nbf),
        "w1big": w1big_dev.astype(nbf),
        "b1big": b1big.reshape(P, 1),
        "wcat2": _wcat2(np.asarray(inputs["ts_w2"], f32),
                        np.asarray(inputs["cls_w2"], f32)).astype(nbf),
        "bcat2": np.concatenate([np.asarray(inputs["ts_b2"], f32),
                                 np.asarray(inputs["cls_b2"], f32),
                                 np.zeros(1, f32)]).reshape(8, 1),
        "b1r": np.tile(np.asarray(inputs["b1"], f32)[None, :], (P, 1)),
        "iota": np.tile(np.arange(P, dtype=f32)[None, :], (P, 1)).astype(nbf),
        "ident": np.eye(P, dtype=f32).astype(nbf),
        "identf": np.eye(P, dtype=f32),
        "ones": np.ones((P, 1), f32),
    }

    tst = np.asarray(inputs["timestamp_target"], np.int64)
    clt = np.asarray(inputs["node_target"], np.int64)
    msk = np.asarray(inputs["node_mask"]).astype(f32)

    in_maps = []
    pos_in_core = node_tile.astype(np.int64) * P + node_slot
    for c in range(NCORES):
        srcw, sendidx, dstloc = per_core[c]
        mine = np.nonzero(node_core == c)[0]
        pos = pos_in_core[mine]
        xT = np.zeros((D_IN, NPAD), f32)
        xT[:, pos] = x[mine].T
        valid = np.zeros(NPAD, bool)
        valid[pos] = True
        g_ts = np.zeros(NPAD, np.int64)
        g_ts[pos] = tst[mine]
        g_cl = np.zeros(NPAD, np.int64)
        g_cl[pos] = clt[mine]
        g_mk = np.zeros(NPAD, f32)
        g_mk[pos] = msk[mine]
        rows = np.arange(NPAD)
        ohts = np.zeros((NPAD, 5), f32)
        ohts[rows, g_ts] = 1.0
        ohcl = np.zeros((NPAD, 2), f32)
        ohcl[rows, g_cl] = 1.0

        def pmf(a, w):
            # [NPAD, w] -> [P, NT*w] (tiles already in schedule order)
            return a.reshape(NT, P, w).transpose(1, 0, 2).reshape(P, NT * w).copy()

        m = {
            "xT": xT.astype(nbf),
            "dstloc": dstloc.astype(nbf),
            "ohts": pmf(ohts, 5),
            "ohcl": pmf(ohcl, 2),
            "vmv": pmf(valid.astype(f32)[:, None], 1),
            "vmm": pmf((g_mk * valid)[:, None], 2 - 1),
        }
        m["srcw"] = srcw
        m["sendidxa"] = sendidx[0]
        m["sendidxb"] = sendidx[1]
        m.update(consts)
        in_maps.append(m)

    cfg = dict(N=N, D_IN=D_IN, NPC=NPC, NT=NT, NPAD=NPAD, TBL=TBL,
               CH=CH, KA=KA, KB=KB, binfo=binfo, PADC=PADC, SCHH=SCHH, NBA=NBA)
    return cfg, in_maps


# ----------------------------------------------------------------------------
# Device graph
# ----------------------------------------------------------------------------

def _build(cfg):
    import os
    STOPAT = int(os.environ.get("STOPAT", "99"))
    N, D_IN = cfg["N"], cfg["D_IN"]
    NT, NPAD, TBL = cfg["NT"], cfg["NPAD"], cfg["TBL"]
    CH = cfg["CH"]
    KA, KB = cfg["KA"], cfg["KB"]
    binfo = cfg["binfo"]
    PADC, SCHH, NBA = cfg["PADC"], cfg["SCHH"], cfg["NBA"]
    RG = [list(range(NCORES))]

    kbmax = max(S * nb for (_, _, nb, S) in binfo)
    NBMAX = max(nb for (_, _, nb, S) in binfo)

    nc = Bacc("TRN2", target_bir_lowering=False, num_devices=NCORES)

    ein = lambda name, shp, dt: nc.dram_tensor(name, shp, dt, kind="ExternalInput")
    xT_d = ein("xT", [D_IN, NPAD], BF16)
    srcw_d = ein("srcw", [P, max(1, CH) * 8], I16)
    sendidx_d = [ein("sendidxa", [P, SCHH[0] * 8], I16),
                 ein("sendidxb", [P, SCHH[1] * 8], I16)]
    dstloc_d = ein("dstloc", [P, CH], BF16)
    ohts_d = ein("ohts", [P, NT * 5], F32)
    ohcl_d = ein("ohcl", [P, NT * 2], F32)
    vmv_d = ein("vmv", [P, NT], F32)
    vmm_d = ein("vmm", [P, NT], F32)
    wtab1_d = ein("wtab1", [D_IN, 80], BF16)
    wsd2_d = ein("wsd2", [64, 16], BF16)
    w1big_d = ein("w1big", [P, 512], BF16)
    b1big_d = ein("b1big", [P, 1], F32)
    wcat2_d = ein("wcat2", [P, 8], BF16)
    bcat2_d = ein("bcat2", [8, 1], F32)
    b1r_d = ein("b1r", [P, 64], F32)
    iota_d = ein("iota", [P, P], BF16)
    identf_d = ein("identf", [P, P], F32)
    ident_d = ein("ident", [P, P], BF16)
    ones_d = ein("ones", [P, 1], F32)

    out_d = nc.dram_tensor("out", [1, 1], F32, kind="ExternalOutput")

    tbl1_loc = nc.dram_tensor("tbl1_loc", [NPAD, RW], BF16)
    tbl2_loc = nc.dram_tensor("tbl2_loc", [NPAD, RW], BF16)
    sb = [[nc.dram_tensor(f"sb{l}{h}", [8 * PADC[h], RW], BF16)
           for h in range(2)] for l in range(2)]
    rb = [[nc.dram_tensor(f"rb{l}{h}", [8 * PADC[h], RW], BF16)
           for h in range(2)] for l in range(2)]
    ar_in = nc.dram_tensor("ar_in", [1, 8], F32)
    ar_out = nc.dram_tensor("ar_out", [1, 8], F32, addr_space="Shared")

    with tile.TileContext(nc) as tc:
        with (
            tc.tile_pool(name="const", bufs=1) as cp,
            tc.tile_pool(name="sbuf", bufs=2) as sp,
            tc.tile_pool(name="stage", bufs=2) as stp,
            tc.tile_pool(name="psum", bufs=2, space="PSUM") as pp,
        ):
            # ---------------- constants to SBUF ----------------
            def ld(t, dram, shape, dt=BF16):
                s = cp.tile(shape, dt, tag=t, name=t)
                nc.sync.dma_start(out=s[: shape[0]], in_=dram[:])
                return s

            wt1a = cp.tile([KA, 80], BF16, tag="wt1a")
            nc.sync.dma_start(out=wt1a[:], in_=wtab1_d[0:KA, :])
            if KB:
                wt1b = cp.tile([max(KB, 32), 80], BF16, tag="wt1b")
                nc.sync.dma_start(out=wt1b[:KB], in_=wtab1_d[KA:D_IN, :])
            wsd2 = ld("wsd2", wsd2_d, [64, 16])
            w1big = ld("w1big", w1big_d, [P, 512])
            b1big = ld("b1big", b1big_d, [P, 1], F32)
            wcat2 = ld("wcat2", wcat2_d, [P, 8])
            bcat2 = ld("bcat2", bcat2_d, [8, 1], F32)
            b1r = ld("b1r", b1r_d, [P, 64], F32)
            iota = ld("iota", iota_d, [P, P])
            ident = ld("ident", ident_d, [P, P])
            identf = ld("identf", identf_d, [P, P], F32)
            ones = ld("ones", ones_d, [P, 1], F32)
            srcw = ld("srcw", srcw_d, [P, max(1, CH) * 8], I16)
            sendidx = [ld("sendidxa", sendidx_d[0], [P, SCHH[0] * 8], I16),
                       ld("sendidxb", sendidx_d[1], [P, SCHH[1] * 8], I16)]
            dstloc = ld("dstloc", dstloc_d, [P, CH])
            ohts = ld("ohts", ohts_d, [P, NT * 5], F32)
            ohcl = ld("ohcl", ohcl_d, [P, NT * 2], F32)
            vmv = ld("vmv", vmv_d, [P, NT], F32)
            vmm = ld("vmm", vmm_d, [P, NT], F32)

            # SBUF-resident local table caches: [feat(64)|a_src(8)|a_dst(8)]
            # per tile, written by phase A (layer 1) / fin1 (layer 2)
            tc1 = cp.tile([P, NT * 80], BF16, tag="tc1")
            tc2 = cp.tile([P, NT * 80], BF16, tag="tc2")

            acc = cp.tile([P, 4], F32, tag="acc")
            nc.vector.memset(acc[:], 0.0)

            # ---------------- phase A: layer-1 table ----------------
            WG = 7  # tiles per table-write group
            for g0 in range(0, NT, WG):
                gn = min(WG, NT - g0)
                xa = sp.tile([P, WG * P], BF16, tag="xa")
                nc.sync.dma_start(out=xa[:, 0:gn * P],
                                  in_=xT_d[0:KA, g0 * P:(g0 + gn) * P])
                if KB:
                    xb = sp.tile([max(KB, 32), WG * P], BF16, tag="xb")
                    nc.sync.dma_start(out=xb[:KB, 0:gn * P],
                                      in_=xT_d[KA:D_IN, g0 * P:(g0 + gn) * P])
                for ti in range(gn):
                    t = g0 + ti
                    pA = pp.tile([P, 512], F32, tag="agg", bufs=2)
                    if KB:
                        nc.tensor.matmul(pA[:, 0:80], lhsT=xa[:, ti * P:(ti + 1) * P],
                                         rhs=wt1a[:], start=True, stop=False)
                        nc.tensor.matmul(pA[:, 0:80], lhsT=xb[:KB, ti * P:(ti + 1) * P],
                                         rhs=wt1b[:KB], start=False, stop=True)
                    else:
                        nc.tensor.matmul(pA[:, 0:80], lhsT=xa[:, ti * P:(ti + 1) * P],
                                         rhs=wt1a[:], start=True, stop=True)
                    nc.scalar.activation(tc1[:, t * 80:(t + 1) * 80], pA[:, 0:80],
                                         AF.Copy)
                tdst = tbl1_loc[:].rearrange("(t p) w -> p t w", p=P)[:, g0:g0 + gn, 0:80]
                nc.sync.dma_start(
                    out=tdst,
                    in_=tc1[:, g0 * 80:(g0 + gn) * 80].rearrange(
                        "p (t w) -> p t w", w=80))

            def halo_exchange(tbl_loc, layer):
                # per half: gather the rows each peer requested into the send
                # buffer, then AllToAll (rank c's shard j -> rank j's shard c).
                # The second half's A2A overlaps the first half's edge compute.
                SGB = 46
                for h in range(2):
                    for p0 in range(0, SCHH[h], SGB):
                        pc = min(SGB, SCHH[h] - p0)
                        gs = sp.tile([P, SGB * RW], BF16, tag="sgb", name="sgb")
                        nc.gpsimd.dma_gather(
                            out_ap=gs[:, 0:pc * RW].rearrange(
                                "p (c e) -> p c e", e=RW),
                            in_ap=tbl_loc[:],
                            idxs_ap=sendidx[h][:, p0 * 8:(p0 + pc) * 8],
                            num_idxs=pc * P, num_idxs_reg=pc * P, elem_size=RW,
                            single_packet=False)
                        nc.sync.dma_start(
                            out=sb[layer][h][:].rearrange(
                                "(c p) e -> p c e", p=P)[:, p0:p0 + pc, :],
                            in_=gs[:, 0:pc * RW].rearrange(
                                "p (c e) -> p c e", e=RW))
                    nc.gpsimd.collective_compute(
                        "AllToAll", OP.bypass, ins=[sb[layer][h][:]],
                        outs=[rb[layer][h][:]], replica_groups=RG,
                    )

            if STOPAT >= 1:
                halo_exchange(tbl1_loc, 0)

            # ---------------- edge phases ----------------
            def edge_layer(layer, tcache, fin_group):
                WM = 72 if layer == 1 else 520
                FW = 64 if layer == 1 else 512
                for bi, (c0, t0, nb, S) in enumerate(binfo):
                    kb = nb * S
                    gm = sp.tile([P, kbmax * RW], BF16, tag="gm")
                    nc.gpsimd.dma_gather(
                        out_ap=gm[:, 0:kb * RW].rearrange("p (c e) -> p c e", e=RW),
                        in_ap=rb[layer - 1][0 if bi < NBA else 1][:],
                        idxs_ap=srcw[:, c0 * 8:(c0 + kb) * 8],
                        num_idxs=kb * P, num_idxs_reg=kb * P, elem_size=RW,
                        single_packet=False)

                    # one-hot [edge, slot] per chunk (window-major dstloc)
                    oh = sp.tile([P, kbmax * P], BF16, tag="oh")
                    nc.vector.tensor_tensor(
                        out=oh[:, 0:kb * P].rearrange("p (c e) -> p c e", e=P),
                        in0=dstloc[:, c0:c0 + kb].unsqueeze(2).to_broadcast(
                            [P, kb, P]),
                        in1=iota[:].unsqueeze(1).to_broadcast([P, kb, P]),
                        op=OP.is_equal,
                    )

                    # transposed one-hots: PE transposes into PSUM slabs,
                    # batched PSUM->SBUF copies, then per-chunk 8-col matmuls
                    # against the local tile's a_dst columns
                    ohT = sp.tile([P, kbmax * P], BF16, tag="ohT")
                    for h0 in range(0, kb, 7):
                        hn = min(7, kb - h0)
                        tpb = pp.tile([P, 7 * P], BF16, tag="tpbB", bufs=2)
                        for i in range(hn):
                            nc.tensor.transpose(tpb[:, i * P:(i + 1) * P],
                                                oh[:, (h0 + i) * P:(h0 + i + 1) * P],
                                                ident[:])
                        nc.scalar.activation(ohT[:, h0 * P:(h0 + hn) * P],
                                             tpb[:, 0:hn * P], AF.Copy)
                    adpe = pp.tile([P, kbmax * 8], F32, tag="adpe", bufs=1)
                    for i_t in range(nb):
                        for q in range(S):
                            jj = i_t * S + q
                            nc.tensor.matmul(
                                adpe[:, jj * 8:(jj + 1) * 8],
                                lhsT=ohT[:, jj * P:(jj + 1) * P],
                                rhs=tcache[:, (t0 + i_t) * 80 + 72:
                                           (t0 + i_t) * 80 + 80],
                                start=True, stop=True)

                    # alpha / leaky relu / exp / weighted messages: one op per
                    # batch (window-major layout is contiguous)
                    alpha = sp.tile([P, kbmax * 8], F32, tag="alpha", bufs=1)
                    msg = sp.tile([P, kbmax * WM], BF16, tag="msg")
                    g4 = gm[:, 0:kb * RW].rearrange("p (c e) -> p c e", e=RW)
                    ms3 = msg[:, 0:kb * WM].rearrange("p (c e) -> p c e", e=WM)
                    nc.vector.tensor_tensor(
                        out=alpha[:, 0:kb * 8].rearrange("p (c e) -> p c e", e=8),
                        in0=g4[:, :, 64:72],
                        in1=adpe[:, 0:kb * 8].rearrange("p (c e) -> p c e", e=8),
                        op=OP.add)
                    nc.vector.scalar_tensor_tensor(
                        out=alpha[:, 0:kb * 8],
                        in0=alpha[:, 0:kb * 8], scalar=0.2,
                        in1=alpha[:, 0:kb * 8], op0=OP.mult, op1=OP.max)
                    # exp straight into the msg tail (denominator columns)
                    nc.scalar.activation(
                        ms3[:, :, WM - 8:WM],
                        alpha[:, 0:kb * 8].rearrange("p (c e) -> p c e", e=8),
                        AF.Exp)
                    if layer == 1:
                        nc.vector.tensor_tensor(
                            out=ms3[:, :, 0:64].rearrange("p c (h z) -> p c h z", h=8),
                            in0=g4[:, :, 0:64].rearrange("p c (h z) -> p c h z", h=8),
                            in1=ms3[:, :, 64:72].unsqueeze(3).to_broadcast(
                                [P, kb, 8, 8]),
                            op=OP.mult,
                        )
                    else:
                        nc.vector.tensor_tensor(
                            out=ms3[:, :, 0:512].rearrange("p c (h z) -> p c h z", h=8),
                            in0=g4[:, :, 0:64].unsqueeze(2).to_broadcast(
                                [P, kb, 8, 64]),
                            in1=ms3[:, :, 512:520].unsqueeze(3).to_broadcast(
                                [P, kb, 8, 64]),
                            op=OP.mult,
                        )

                    # per-tile aggregation + PSUM->SBUF copy into group slabs
                    FWW = FW + (8 if layer == 1 else 0)
                    pzs = sp.tile([P, NBMAX * FWW], F32 if layer == 1 else BF16,
                                  tag=f"pzs{layer}", bufs=1,
                                  name="pzs")
                    pds = (sp.tile([P, NBMAX * 8], F32, tag="pds", bufs=1, name="pds")
                           if layer == 2 else None)
                    pdp = (pp.tile([P, kbmax * 8], F32, tag="adpe", bufs=1,
                                   name="pdp")
                           if layer == 2 else None)
                    for i_t in range(nb):
                        pz = pp.tile([P, 512], F32, tag="agg", bufs=2, name="pz")
                        pd = (pdp[:, i_t * 8:(i_t + 1) * 8]
                              if layer == 2 else None)
                        for q in range(S):
                            jj = i_t * S + q
                            ohj = oh[:, jj * P:(jj + 1) * P]
                            mj = msg[:, jj * WM:(jj + 1) * WM]
                            st, fi = (q == 0), (q == S - 1)
                            nc.tensor.matmul(pz[:, 0:FWW], lhsT=ohj,
                                             rhs=mj[:, 0:FWW],
                                             start=st, stop=fi)
                            if layer == 2:
                                nc.tensor.matmul(pd[:], lhsT=ohj,
                                                 rhs=mj[:, 512:520],
                                                 start=st, stop=fi)
                        nc.scalar.activation(pzs[:, i_t * FWW:(i_t + 1) * FWW],
                                             pz[:, 0:FWW], AF.Copy)
                        if layer == 2:
                            nc.vector.tensor_copy(pds[:, i_t * 8:(i_t + 1) * 8],
                                                  pd[:])
                    fin_group(t0, nb, pzs, pds)

            # ---------------- group finalizers ----------------
            def selfloop_ea_grp(tcache, t0, nb):
                # ea of each node's own self-loop: exp(lrelu(a_src + a_dst))
                tg = tcache[:, t0 * 80:(t0 + nb) * 80].rearrange(
                    "p (t w) -> p t w", w=80)
                asum = sp.tile([P, NBMAX * 8], F32, tag="asum", bufs=1)
                nc.vector.tensor_tensor(
                    out=asum[:, 0:nb * 8].rearrange("p (t e) -> p t e", e=8),
                    in0=tg[:, :, 64:72], in1=tg[:, :, 72:80], op=OP.add)
                lrs = sp.tile([P, NBMAX * 8], F32, tag="lrs", bufs=1)
                nc.vector.scalar_tensor_tensor(
                    out=lrs[:, 0:nb * 8], in0=asum[:, 0:nb * 8], scalar=0.2,
                    in1=asum[:, 0:nb * 8], op0=OP.mult, op1=OP.max)
                eas = sp.tile([P, NBMAX * 8], F32, tag="eas", bufs=1)
                nc.scalar.activation(eas[:, 0:nb * 8], lrs[:, 0:nb * 8], AF.Exp)
                return eas

            t2_state = {"n": 0}

            def fin1_group(gt0, gnb, gpzs, gpds):
              for o0 in range(0, gnb, 4):
                nb = min(4, gnb - o0)
                t0 = gt0 + o0
                pzs = gpzs[:, o0 * 72:(o0 + nb) * 72]
                # pzs: [P, nb*72] = [num(64) | denom(8)] per tile
                pz3 = pzs[:, 0:nb * 72].rearrange("p (t e) -> p t e", e=72)
                tg = tc1[:, t0 * 80:(t0 + nb) * 80].rearrange(
                    "p (t w) -> p t w", w=80)
                eas = selfloop_ea_grp(tc1, t0, nb)
                ea3 = eas[:, 0:nb * 8].rearrange("p (t e) -> p t e", e=8)
                rin = sp.tile([P, NBMAX * 8], F32, tag="rin", bufs=1)
                nc.vector.scalar_tensor_tensor(
                    out=rin[:, 0:nb * 8].rearrange("p (t e) -> p t e", e=8),
                    in0=pz3[:, :, 64:72], scalar=1e-16, in1=ea3,
                    op0=OP.add, op1=OP.add)
                rcp = sp.tile([P, NBMAX * 8], F32, tag="rcp", bufs=1)
                nc.vector.reciprocal(rcp[:, 0:nb * 8], rin[:, 0:nb * 8])
                num = sp.tile([P, NBMAX * 64], F32, tag="num", bufs=1)
                nc.vector.tensor_tensor(
                    out=num[:, 0:nb * 64].rearrange("p (t h c) -> p t h c", h=8, c=8),
                    in0=tg[:, :, 0:64].rearrange("p t (h c) -> p t h c", h=8),
                    in1=ea3.unsqueeze(3).to_broadcast([P, nb, 8, 8]),
                    op=OP.mult,
                )
                nc.vector.tensor_tensor(
                    out=num[:, 0:nb * 64].rearrange("p (t e) -> p t e", e=64),
                    in0=num[:, 0:nb * 64].rearrange("p (t e) -> p t e", e=64),
                    in1=pz3[:, :, 0:64], op=OP.add)
                h1f = sp.tile([P, NBMAX * 64], F32, tag="h1f", bufs=1)
                nc.vector.tensor_tensor(
                    out=h1f[:, 0:nb * 64].rearrange("p (t h c) -> p t h c", h=8, c=8),
                    in0=num[:, 0:nb * 64].rearrange("p (t h c) -> p t h c", h=8, c=8),
                    in1=rcp[:, 0:nb * 8].rearrange("p (t e) -> p t e", e=8)
                        .unsqueeze(3).to_broadcast([P, nb, 8, 8]),
                    op=OP.mult,
                )
                # layer-2 table rows: feat = h1f + b1, attn via wsd2
                trow = stp.tile([P, NBMAX * RW], BF16, tag="tbl2_w", name="tbl2w")
                nc.vector.tensor_tensor(
                    out=trow[:, 0:nb * RW].rearrange(
                        "p (t e) -> p t e", e=RW)[:, :, 0:64],
                    in0=h1f[:, 0:nb * 64].rearrange("p (t e) -> p t e", e=64),
                    in1=b1r[:].unsqueeze(1).to_broadcast([P, nb, 64]),
                    op=OP.add)
                # transposes of the nb feature blocks + one batched copy
                tpb = pp.tile([P, 7 * P], BF16, tag="tpbB", bufs=2)
                for i_t in range(nb):
                    nc.tensor.transpose(
                        tpb[0:64, i_t * P:(i_t + 1) * P],
                        trow[:, i_t * RW:i_t * RW + 64], ident[:])
                h1T = sp.tile([64, 7 * P], BF16, tag="h1T", bufs=1)
                nc.scalar.activation(h1T[:, 0:nb * P], tpb[0:64, 0:nb * P], AF.Copy)
                pf = pp.tile([P, NBMAX * 64], F32, tag="hp", bufs=1)
                for i_t in range(nb):
                    nc.tensor.matmul(pf[:, i_t * 16:(i_t + 1) * 16],
                                     lhsT=h1T[:, i_t * P:(i_t + 1) * P],
                                     rhs=wsd2[:], start=True, stop=True)
                nc.scalar.activation(
                    trow[:, 0:nb * RW].rearrange("p (t e) -> p t e", e=RW)[:, :, 64:80],
                    pf[:, 0:nb * 16].rearrange("p (t e) -> p t e", e=16), AF.Copy)
                nc.vector.tensor_copy(
                    tc2[:, t0 * 80:(t0 + nb) * 80].rearrange(
                        "p (t e) -> p t e", e=80),
                    trow[:, 0:nb * RW].rearrange("p (t e) -> p t e", e=RW)[:, :, 0:80])
                tdst = tbl2_loc[:].rearrange("(t p) w -> p t w", p=P)[
                    :, t0:t0 + nb, 0:80]
                nc.sync.dma_start(
                    out=tdst,
                    in_=trow[:, 0:nb * RW].rearrange(
                        "p (t e) -> p t e", e=RW)[:, :, 0:80])
                t2_state["n"] += nb

            if STOPAT >= 2:
                edge_layer(1, tc1, fin1_group)

            if STOPAT >= 3:
                halo_exchange(tbl2_loc, 1)

            # ---------------- layer-2 finalize: h2, MLPs, CE ----------------
            ceall_ts = cp.tile([P, NT], F32, tag="cets")
            ceall_cl = cp.tile([P, NT], F32, tag="cecl")
            sum2 = cp.tile([P, 2 * NT], F32, tag="sum2")   # [ts | cl] exp-sums
            pk2 = cp.tile([P, 2 * NT], F32, tag="pk2")     # picked logits

            def fin2_group(gt0, gnb, gpzs, gpds):
              for o0 in range(0, gnb, 4):
                nb = min(4, gnb - o0)
                t0 = gt0 + o0
                pzs = gpzs[:, o0 * 512:(o0 + nb) * 512]
                pds = gpds[:, o0 * 8:(o0 + nb) * 8]
                # pzs: [P, nb*512] per-head numerators; pds: [P, nb*8] denoms
                tg = tc2[:, t0 * 80:(t0 + nb) * 80].rearrange(
                    "p (t w) -> p t w", w=80)
                eas = selfloop_ea_grp(tc2, t0, nb)
                ea3 = eas[:, 0:nb * 8].rearrange("p (t e) -> p t e", e=8)
                rin = sp.tile([P, NBMAX * 8], F32, tag="rin", bufs=1)
                nc.vector.scalar_tensor_tensor(
                    out=rin[:, 0:nb * 8].rearrange("p (t e) -> p t e", e=8),
                    in0=pds[:, 0:nb * 8].rearrange("p (t e) -> p t e", e=8),
                    scalar=1e-16, in1=ea3, op0=OP.add, op1=OP.add)
                rcp = sp.tile([P, NBMAX * 8], F32, tag="rcp", bufs=1)
                nc.vector.reciprocal(rcp[:, 0:nb * 8], rin[:, 0:nb * 8])
                num = sp.tile([P, NBMAX * 512], F32, tag="num2", bufs=1)
                nc.vector.tensor_tensor(
                    out=num[:, 0:nb * 512].rearrange(
                        "p (t h c) -> p t h c", h=8, c=64),
                    in0=tg[:, :, 0:64].unsqueeze(2).to_broadcast([P, nb, 8, 64]),
                    in1=ea3.unsqueeze(3).to_broadcast([P, nb, 8, 64]),
                    op=OP.mult,
                )
                nc.vector.tensor_tensor(
                    out=num[:, 0:nb * 512],
                    in0=num[:, 0:nb * 512], in1=pzs[:, 0:nb * 512], op=OP.add)
                zn = sp.tile([P, NBMAX * 512], BF16, tag="zn", bufs=1)
                nc.vector.tensor_tensor(
                    out=zn[:, 0:nb * 512].rearrange(
                        "p (t h c) -> p t h c", h=8, c=64),
                    in0=num[:, 0:nb * 512].rearrange(
                        "p (t h c) -> p t h c", h=8, c=64),
                    in1=rcp[:, 0:nb * 8].rearrange("p (t e) -> p t e", e=8)
                        .unsqueeze(3).to_broadcast([P, nb, 8, 64]),
                    op=OP.mult,
                )
                # k-major transposes of zn; composed (wbig @ mlp-w1) matmul
                nblk = nb * 4
                zT = sp.tile([P, NBMAX * 4 * P], BF16, tag="zT", bufs=1)
                pos = 0
                tpb = None
                for k in range(4):
                    for i_t in range(nb):
                        if pos % 7 == 0:
                            if pos:
                                nc.scalar.activation(
                                    zT[:, (pos - 7) * P:pos * P],
                                    tpb[:, 0:7 * P], AF.Copy)
                            tpb = pp.tile([P, 7 * P], BF16, tag="tpbB", bufs=2)
                        nc.tensor.transpose(
                            tpb[:, (pos % 7) * P:(pos % 7 + 1) * P],
                            zn[:, (i_t * 4 + k) * P:(i_t * 4 + k + 1) * P],
                            ident[:])
                        pos += 1
                rem = pos % 7 or 7
                nc.scalar.activation(zT[:, (pos - rem) * P:pos * P],
                                     tpb[:, 0:rem * P], AF.Copy)
                pw = nb * P
                pa = pp.tile([P, 7 * P], F32, tag="tp", bufs=1, name="pa")
                for k in range(4):
                    nc.tensor.matmul(pa[:, 0:pw], lhsT=w1big[:, k * P:(k + 1) * P],
                                     rhs=zT[:, k * nb * P:(k * nb + nb) * P],
                                     start=(k == 0), stop=(k == 3))
                h12T = sp.tile([P, 7 * P], BF16, tag="h12T", bufs=1)
                nc.scalar.activation(h12T[:, 0:pw], pa[:, 0:pw], AF.Relu,
                                     bias=b1big[:, 0:1])
                lgx = pp.tile([P, 7 * P], F32, tag="tp", bufs=1, name="lg")
                lg = lgx[0:8]
                nc.tensor.matmul(lg[0:8, 0:pw], lhsT=wcat2[:],
                                 rhs=h12T[:, 0:pw], start=True, stop=True)
                lgsm = sp.tile([8, 7 * P], F32, tag="lgsm", bufs=1)
                nc.scalar.activation(lgsm[0:8, 0:pw], lg[0:8, 0:pw],
                                     AF.Identity, bias=bcat2[0:8, 0:1])
                ptlx = pp.tile([P, NBMAX * 64], F32, tag="hp", bufs=1,
                               name="ptl")
                ptl = ptlx[:, 0:NBMAX * 8]
                for i_t in range(nb):
                    nc.tensor.matmul(ptl[:, i_t * 8:(i_t + 1) * 8],
                                     lhsT=lgsm[0:8, i_t * P:(i_t + 1) * P],
                                     rhs=identf[0:8, 0:8], is_transpose=True,
                                     start=True, stop=True)
                # batched CE over the group's tiles
                tl3 = ptl[:, 0:nb * 8].rearrange("p (t e) -> p t e", e=8)
                ex_ts = sp.tile([P, NBMAX * 5], F32, tag="exts", bufs=1)
                ex_cl = sp.tile([P, NBMAX * 2], F32, tag="excl", bufs=1)
                nc.scalar.activation(
                    ex_ts[:, 0:nb * 5].rearrange("p (t e) -> p t e", e=5),
                    tl3[:, :, 0:5], AF.Exp)
                nc.scalar.activation(
                    ex_cl[:, 0:nb * 2].rearrange("p (t e) -> p t e", e=2),
                    tl3[:, :, 5:7], AF.Exp)
                nc.vector.reduce_sum(
                    sum2[:, t0:t0 + nb].rearrange("p (t e) -> p t e", e=1),
                    ex_ts[:, 0:nb * 5].rearrange("p (t e) -> p t e", e=5),
                    axis=mybir.AxisListType.X)
                nc.vector.reduce_sum(
                    sum2[:, NT + t0:NT + t0 + nb].rearrange("p (t e) -> p t e", e=1),
                    ex_cl[:, 0:nb * 2].rearrange("p (t e) -> p t e", e=2),
                    axis=mybir.AxisListType.X)
                pk_ts = sp.tile([P, NBMAX * 5], F32, tag="pkts", bufs=1)
                pk_cl = sp.tile([P, NBMAX * 2], F32, tag="pkcl", bufs=1)
                nc.vector.tensor_tensor(
                    out=pk_ts[:, 0:nb * 5].rearrange("p (t e) -> p t e", e=5),
                    in0=tl3[:, :, 0:5],
                    in1=ohts[:, t0 * 5:(t0 + nb) * 5].rearrange(
                        "p (t e) -> p t e", e=5), op=OP.mult)
                nc.vector.tensor_tensor(
                    out=pk_cl[:, 0:nb * 2].rearrange("p (t e) -> p t e", e=2),
                    in0=tl3[:, :, 5:7],
                    in1=ohcl[:, t0 * 2:(t0 + nb) * 2].rearrange(
                        "p (t e) -> p t e", e=2), op=OP.mult)
                nc.vector.reduce_sum(
                    pk2[:, t0:t0 + nb].rearrange("p (t e) -> p t e", e=1),
                    pk_ts[:, 0:nb * 5].rearrange("p (t e) -> p t e", e=5),
                    axis=mybir.AxisListType.X)
                nc.vector.reduce_sum(
                    pk2[:, NT + t0:NT + t0 + nb].rearrange("p (t e) -> p t e", e=1),
                    pk_cl[:, 0:nb * 2].rearrange("p (t e) -> p t e", e=2),
                    axis=mybir.AxisListType.X)

            if STOPAT >= 4:
                edge_layer(2, tc2, fin2_group)
                lse2 = cp.tile([P, 2 * NT], F32, tag="lse2")
                nc.scalar.activation(lse2[:], sum2[:], AF.Ln)
                nc.vector.tensor_sub(lse2[:], lse2[:], pk2[:])
                nc.vector.tensor_tensor(out=ceall_ts[:], in0=lse2[:, 0:NT],
                                        in1=vmv[:], op=OP.mult)
                nc.vector.tensor_tensor(out=ceall_cl[:], in0=lse2[:, NT:2 * NT],
                                        in1=vmm[:], op=OP.mult)
                nc.vector.reduce_sum(acc[:, 0:1], ceall_ts[:],
                                     axis=mybir.AxisListType.X)
                nc.vector.reduce_sum(acc[:, 1:2], ceall_cl[:],
                                     axis=mybir.AxisListType.X)
                nc.vector.reduce_sum(acc[:, 2:3], vmm[:],
                                     axis=mybir.AxisListType.X)

            # ---------------- final reduction ----------------
            pfinx = pp.tile([P, 7 * P], F32, tag="tp", bufs=1)
            pfin = pfinx[0:1, 0:8]
            nc.tensor.matmul(pfin[0:1, 0:3], lhsT=ones[:], rhs=acc[:, 0:3],
                             start=True, stop=True)
            fin_sb = cp.tile([1, 8], F32, tag="fin")
            nc.vector.memset(fin_sb[:], 0.0)
            nc.scalar.activation(fin_sb[0:1, 0:3], pfin[0:1, 0:3], AF.Copy)
            nc.sync.dma_start(out=ar_in[:], in_=fin_sb[:])
            nc.gpsimd.collective_compute(
                "AllReduce", OP.add, ins=[ar_in[:]], outs=[ar_out[:]],
                replica_groups=RG,
            )
            tot = cp.tile([1, 8], F32, tag="tot")
            nc.sync.dma_start(out=tot[:], in_=ar_out[:])
            rcpm = cp.tile([1, 1], F32, tag="rcpm")
            nc.vector.reciprocal(rcpm[:], tot[:, 2:3])
            lcl = cp.tile([1, 1], F32, tag="lcl")
            nc.vector.tensor_tensor(out=lcl[:], in0=tot[:, 1:2], in1=rcpm[:], op=OP.mult)
            lts = cp.tile([1, 1], F32, tag="lts")
            nc.vector.tensor_scalar_mul(lts[:], tot[:, 0:1], 1.0 / N)
            res = cp.tile([1, 1], F32, tag="res")
            nc.vector.tensor_add(res[:], lcl[:], lts[:])
            nc.sync.dma_start(out=out_d[:], in_=res[:])

    nc.compile()
    return nc


# ----------------------------------------------------------------------------
# Entry points
# ----------------------------------------------------------------------------

def _run(inputs, trace=False):
    cfg, in_maps = _prep(inputs)
    nc = _build(cfg)
    try:
        r = run_bass_kernel_spmd(nc, in_maps, core_ids=list(range(NCORES)), trace=trace)
    except ModuleNotFoundError:
        r = run_bass_kernel_spmd(nc, in_maps, core_ids=list(range(NCORES)), trace=False)
    out = np.asarray(r.results[0]["out"], np.float32).reshape(())
    return out, r


def kernel(**inputs):
    out, _ = _run(inputs, trace=False)
    return out


def _build_null(cfg):
    """Same I/O signature, trivial compute — for dispatch/transfer baseline."""
    N, D_IN = cfg["N"], cfg["D_IN"]
    NT, NPAD, TBL = cfg["NT"], cfg["NPAD"], cfg["TBL"]
    CH = cfg["CH"]
    nc = Bacc("TRN2", target_bir_lowering=False, num_devices=NCORES)
    ein = lambda name, shp, dt: nc.dram_tensor(name, shp, dt, kind="ExternalInput")
    xT_d = ein("xT", [D_IN, NPAD], BF16)
    ein("srcw", [P, max(1, CH) * 8], I16)
    ein("sendidxa", [P, cfg["SCHH"][0] * 8], I16)
    ein("sendidxb", [P, cfg["SCHH"][1] * 8], I16)
    ein("dstloc", [P, CH], BF16)
    ein("ohts", [P, NT * 5], F32)
    ein("ohcl", [P, NT * 2], F32)
    ein("vmv", [P, NT], F32)
    ein("vmm", [P, NT], F32)
    ein("wtab1", [D_IN, 80], BF16)
    ein("wsd2", [64, 16], BF16)
    ein("w1big", [P, 512], BF16)
    ein("b1big", [P, 1], F32)
    ein("wcat2", [P, 8], BF16)
    ein("bcat2", [8, 1], F32)
    ein("b1r", [P, 64], F32)
    ein("iota", [P, P], BF16)
    identf_d = ein("identf", [P, P], F32)
    ein("ident", [P, P], BF16)
    ein("ones", [P, 1], F32)
    out_d = nc.dram_tensor("out", [1, 1], F32, kind="ExternalOutput")
    with tile.TileContext(nc) as tc:
        with tc.tile_pool(name="sp", bufs=1) as sp:
            t = sp.tile([1, 1], F32, tag="t")
            nc.sync.dma_start(out=t[:], in_=identf_d[0:1, 0:1])
            nc.sync.dma_start(out=out_d[:], in_=t[:])
    nc.compile()
    return nc
